# revision 1
# baseline (speedup 1.0000x reference)
"""v2 feature kernel: dual-player packed planes (my: bits 0-7, op: bits 16-23)
and column-oriented line features computed in the row-byte layout via
row-offset access patterns. Halves the boolean-logic op count of v1 and
removes the column packing + merge expansion passes.

Plane geometry: padded tiles [P, NB, 18] u32, valid rows 5:13, 5 guard rows
of zeros on each side (column windows reach +-5 rows).
"""
import numpy as np

import concourse.bass as bass
import concourse.bacc as bacc
import concourse.mybir as mybir
import concourse.tile as tile

Alu = mybir.AluOpType
Act = mybir.ActivationFunctionType
DT = mybir.dt

P = 128
NB = 32
CB = 4
NCHUNK = NB // CB
NCORES = 8
BPC = P * NB
PAD = 18
R0 = 5
ROWS = slice(R0, R0 + 8)
SEG = 0x00FF00FF  # both player segments

DIRS = ((0, 1), (1, 0), (1, 1), (1, -1))


def _build_masks() -> np.ndarray:
    """[P, 2*64] u32: lane masks for my (1<<j) and op (1<<(16+j))."""
    j = np.tile(np.arange(8), 8)
    t = np.concatenate([(1 << j), (1 << (16 + j))]).reshape(1, 128).astype(np.uint32)
    return np.broadcast_to(t, (P, 128)).copy()


def _stt_raw(eng, out, in0, imm, in1, op0, op1, imm_dt=DT.uint32):
    outs = [eng.lower_ap(out)]
    return eng.add_instruction(
        mybir.InstTensorScalarPtr(
            name=eng.bass.get_next_instruction_name(),
            is_scalar_tensor_tensor=True,
            op0=op0, op1=op1,
            ins=[eng.lower_ap(in0),
                 mybir.ImmediateValue(dtype=imm_dt, value=imm),
                 eng.lower_ap(in1)],
            outs=outs,
        )
    )


def _stt(eng, out, in0, sh, op1, in1):
    if sh > 0:
        _stt_raw(eng, out, in0, sh, in1, Alu.logical_shift_left, op1)
    elif sh < 0:
        _stt_raw(eng, out, in0, -sh, in1, Alu.logical_shift_right, op1)
    else:
        eng.tensor_tensor(out, in0, in1, op1)


def feature_kernel(tc, out_d, state_d, side_d):
    nc = tc.nc
    V, G, A = nc.vector, nc.gpsimd, nc.scalar

    state_v = state_d.rearrange("(p n) c -> p n c", p=P)
    side_v = side_d.rearrange("(p n) -> p n", p=P)
    out_v = out_d.rearrange("(p n) c -> p n c", p=P)

    with (
        tc.tile_pool(name="main", bufs=1) as pool,
        tc.tile_pool(name="chk", bufs=2) as cpool,
    ):
        # ---------- input ----------
        sideT = pool.tile([P, NB], DT.float32, name="sideT")
        nc.sync.dma_start(sideT[:], side_v)
        # expansion bit masks built on-device: 1<<j per cell, and <<16 for op
        jv = pool.tile([P, 64], DT.uint32, name="jv")
        G.iota(jv[:], pattern=[[0, 8], [1, 8]], base=0, channel_multiplier=0)
        onesp = pool.tile([P, 64], DT.uint32, name="onesp")
        V.memset(onesp[:], 1)
        masks = pool.tile([P, 128], DT.uint32, name="masks")
        V.tensor_tensor(masks[:, 0:64], onesp[:], jv[:], Alu.logical_shift_left)
        V.tensor_scalar(masks[:, 64:128], masks[:, 0:64], 16, None,
                        Alu.logical_shift_left)

        s = pool.tile([P, NB, 64], DT.float32, name="s")
        nc.sync.dma_start(s[:], state_v)
        negside = pool.tile([P, NB], DT.float32, name="negside")
        V.tensor_scalar(negside[:], sideT[:], -1.0, None, Alu.mult)
        myf = pool.tile([P, NB, 64], DT.float32, name="myf")
        opf = pool.tile([P, NB, 64], DT.float32, name="opf")
        V.tensor_tensor(
            myf[:], s[:], sideT[:, :, None].broadcast_to((P, NB, 64)), Alu.is_equal
        )
        V.tensor_tensor(
            opf[:], s[:], negside[:, :, None].broadcast_to((P, NB, 64)), Alu.is_equal
        )

        # ---------- padded plane allocator ----------
        def ptile(name, lo=1, hi=5):
            """Padded tile; zero only guard rows [R0-lo, R0) and [13, 13+hi)."""
            t = pool.tile([P, NB, PAD], DT.uint32, name=name)
            V.memset(t[:, :, R0 - lo:R0], 0)
            V.memset(t[:, :, R0 + 8:R0 + 8 + hi], 0)
            return t

        # ---------- packing ----------
        myR = pool.tile([P, NB, 8], DT.uint32, name="myR")
        opR = pool.tile([P, NB, 8], DT.uint32, name="opR")

        def pack(dst_ap, srcf):
            v = srcf.rearrange("p n (r j2 t) -> p (n r) j2 t", t=2, j2=4)
            a1, b1 = v[:, :, :, 1], v[:, :, :, 0]
            t1 = pool.tile([P, NB * 8, 4], DT.float32, name="pk_t1")
            V.scalar_tensor_tensor(t1[:], a1, 2.0, b1, op0=Alu.mult, op1=Alu.add)
            w2 = t1.rearrange("p q (k t) -> p q k t", t=2)
            a2, b2 = w2[:, :, :, 1], w2[:, :, :, 0]
            t2 = pool.tile([P, NB * 8, 2], DT.float32, name="pk_t2")
            V.scalar_tensor_tensor(t2[:], a2, 4.0, b2, op0=Alu.mult, op1=Alu.add)
            w3 = t2.rearrange("p (n r) t -> p n r t", r=8)
            a3, b3 = w3[:, :, :, 1], w3[:, :, :, 0]
            V.scalar_tensor_tensor(dst_ap, a3, 16.0, b3, op0=Alu.mult, op1=Alu.add)

        pack(myR[:], myf)
        pack(opR[:], opf)

        # dual planes: Ad = my | op<<16 ; Bd = op | my<<16 (op from my-persp etc.)
        Ad = ptile("Ad")
        Bd = ptile("Bd")
        _stt(V, Ad[:, :, ROWS], opR[:], 16, Alu.bitwise_or, myR[:])
        _stt(V, Bd[:, :, ROWS], myR[:], 16, Alu.bitwise_or, opR[:])
        Ed = ptile("Ed")      # empty (same in both segments)
        V.tensor_tensor(Ed[:, :, ROWS], Ad[:, :, ROWS], Bd[:, :, ROWS], Alu.bitwise_or)
        V.tensor_scalar(Ed[:, :, ROWS], Ed[:, :, ROWS], SEG, None, Alu.bitwise_xor)
        Nd = ptile("Nd")      # notme of Ad
        V.tensor_scalar(Nd[:, :, ROWS], Ad[:, :, ROWS], SEG, None, Alu.bitwise_xor)

        # channel plane group (dual): [c1 c2 c3 l2 l3 r3]
        Rg = pool.tile([P, 6, NB, 8], DT.uint32, name="Rg")

        # ---------- connectivity (dual, 4 dirs) ----------
        d2 = ptile("cn_d2", 1, 2); d3 = ptile("cn_d3", 1, 2); d4 = ptile("cn_d4", 1, 2)
        t3 = ptile("cn_t3", 1, 2); t4 = ptile("cn_t4", 1, 2)
        Atiles = {}
        for di_i in range(4):
            for N in (2, 3, 4):
                Atiles[(di_i, N)] = pool.tile([P, NB, 8], DT.uint32,
                                              name=f"cn_a{N}_{di_i}")

        def AV(di_i, N):
            return Atiles[(di_i, N)][:]

        cx1 = pool.tile([P, NB, 8], DT.uint32, name="cx1")
        cx2 = pool.tile([P, NB, 8], DT.uint32, name="cx2")
        cx3 = pool.tile([P, NB, 8], DT.uint32, name="cx3")

        mv = Ad[:, :, ROWS]
        for di_i, (di, dj) in enumerate(DIRS):
            def fwd(t):
                return t[:, :, R0 - di:R0 + 8 - di]

            def bwd(t, k=1):
                return t[:, :, R0 + k * di:R0 + 8 + k * di]

            a2, a3, a4 = (AV(di_i, N) for N in (2, 3, 4))
            _stt(V, d2[:, :, ROWS], fwd(Ad), dj, Alu.bitwise_and, mv)
            _stt(V, d3[:, :, ROWS], fwd(d2), dj, Alu.bitwise_and, d2[:, :, ROWS])
            _stt(V, d4[:, :, ROWS], fwd(d3), dj, Alu.bitwise_and, d3[:, :, ROWS])
            _stt(V, a2, bwd(d2), -dj, Alu.bitwise_or, d2[:, :, ROWS])
            _stt(V, t3[:, :, ROWS], bwd(d3), -dj, Alu.bitwise_or, d3[:, :, ROWS])
            _stt(V, a3, bwd(d3, 2), -2 * dj, Alu.bitwise_or, t3[:, :, ROWS])
            _stt(V, t4[:, :, ROWS], bwd(d4), -dj, Alu.bitwise_or, d4[:, :, ROWS])
            _stt(V, a4, bwd(t4, 2), -2 * dj, Alu.bitwise_or, t4[:, :, ROWS])

        V.tensor_tensor(cx1[:], AV(0, 2), AV(1, 2), Alu.bitwise_and)
        V.tensor_tensor(cx1[:], cx1[:], AV(2, 2), Alu.bitwise_and)
        V.tensor_tensor(cx1[:], cx1[:], AV(3, 2), Alu.bitwise_and)
        V.tensor_tensor(Rg[:, 0], mv, cx1[:], Alu.bitwise_xor)
        for k, N in ((1, 2), (2, 3)):
            V.tensor_tensor(cx1[:], AV(0, N), AV(0, N + 1), Alu.bitwise_xor)
            V.tensor_tensor(cx2[:], AV(1, N), AV(1, N + 1), Alu.bitwise_xor)
            V.tensor_tensor(cx1[:], cx1[:], cx2[:], Alu.bitwise_or)
            V.tensor_tensor(cx2[:], AV(2, N), AV(2, N + 1), Alu.bitwise_xor)
            V.tensor_tensor(cx3[:], AV(3, N), AV(3, N + 1), Alu.bitwise_xor)
            V.tensor_tensor(cx2[:], cx2[:], cx3[:], Alu.bitwise_or)
            V.tensor_tensor(Rg[:, k], cx1[:], cx2[:], Alu.bitwise_or)

        # ---------- line features ----------
        # padded tmp tiles (shared by row/col calls; guards stay zero)
        TMP = {}
        R0T = 2

        def tp(name):
            if name not in TMP:
                t = pool.tile([P, NB, 12], DT.uint32, name="lf_" + name)
                blocks = t.rearrange("p n (a b) -> p n a b", a=6, b=2)
                V.memset(blocks[:, :, 0::5, :], 0)   # rows 0:2 and 10:12
                TMP[name] = t
            return TMP[name]

        lf_row = {}  # row-mode results: l2 (final bits), l3 (bits unshifted), r3

        def line_feats_row():
            me, op, em, nm = (x[:, :, ROWS] for x in (Ad, Bd, Ed, Nd))

            def T(n):
                return tp(n)[:, :, R0T:R0T + 8]

            _stt(V, T("t"), me, -1, Alu.bitwise_and, me)
            _stt(V, T("u"), em, -1, Alu.bitwise_and, em)
            _stt(V, T("a"), T("u"), -2, Alu.bitwise_and, T("t"))
            _stt(V, T("w"), em, -3, Alu.bitwise_and, em)
            _stt(V, T("b"), T("t"), -1, Alu.bitwise_and, T("w"))
            _stt(V, T("y"), T("b"), 1, Alu.bitwise_or, T("b"))
            V.tensor_tensor(T("q"), T("a"), T("y"), Alu.bitwise_or)
            _stt(V, T("l2"), T("q"), 1, Alu.bitwise_or, T("a"))

            _stt(V, T("m3"), me, -2, Alu.bitwise_and, T("t"))
            _stt(V, T("r1"), em, -4, Alu.bitwise_and, em)
            _stt(V, T("c"), T("m3"), -1, Alu.bitwise_and, T("r1"))
            _stt(V, T("i1"), T("c"), 1, Alu.bitwise_or, T("c"))
            _stt(V, T("l3"), T("i1"), 1, Alu.bitwise_or, T("c"))  # unshifted

            V.tensor_scalar(T("lb"), op, 1, 0x00010001,
                            op0=Alu.logical_shift_left, op1=Alu.bitwise_or)
            _stt(V, T("d0"), em, -3, Alu.bitwise_and, T("m3"))
            _stt(V, T("d1"), nm, -4, Alu.bitwise_and, T("d0"))
            V.tensor_tensor(T("d"), T("d1"), T("lb"), Alu.bitwise_and)
            _stt(V, T("j1"), T("d"), 1, Alu.bitwise_or, T("d"))
            _stt(V, T("md"), T("d"), 2, Alu.bitwise_or, T("j1"))
            _stt(V, T("o3"), T("m3"), -1, Alu.bitwise_and, nm)
            _stt(V, T("o3"), nm, -4, Alu.bitwise_and, T("o3"))
            V.tensor_scalar(T("rb"), op, 5, 0x00F800F8,
                            op0=Alu.logical_shift_right, op1=Alu.bitwise_or)
            V.tensor_tensor(T("x"), T("lb"), T("rb"), Alu.bitwise_xor)
            V.tensor_tensor(T("e"), T("o3"), T("x"), Alu.bitwise_and)
            _stt(V, T("g1"), T("e"), 1, Alu.bitwise_or, T("e"))
            _stt(V, T("g2"), T("g1"), 1, Alu.bitwise_or, T("e"))
            _stt(V, T("r3"), T("g2"), 1, Alu.bitwise_or, T("md"))
            lf_row["l2"] = tp("l2"); lf_row["l3"] = tp("l3"); lf_row["r3"] = tp("r3")

        line_feats_row()

        # column mode: positions along rows; shifts become row-offset views.
        # up_k(x): value from k rows earlier (toward row 0); dn_k: k rows later.
        lbmC = pool.tile([P, 8], DT.uint32, name="lbmC")
        rbmC = pool.tile([P, 8], DT.uint32, name="rbmC")
        V.memset(lbmC[:], 0)
        V.memset(lbmC[:, 0:1], SEG)
        V.memset(rbmC[:], 0)
        V.memset(rbmC[:, 3:8], SEG)

        def line_feats_col():
            def dn(x, k):  # x[r+k]
                return x[:, :, R0 + k:R0 + 8 + k]

            def T(n, k=0):
                nm = "c_" + n if n in ("l2", "l3", "r3") else n
                t = tp(nm)
                return t[:, :, R0T + k:R0T + 8 + k]

            me, op, em, nm = Ad, Bd, Ed, Nd

            def MV(x, k=0):
                return x[:, :, R0 + k:R0 + 8 + k]

            V.tensor_tensor(T("t"), MV(me), dn(me, 1), Alu.bitwise_and)
            V.tensor_tensor(T("u"), MV(em), dn(em, 1), Alu.bitwise_and)
            V.tensor_tensor(T("a"), T("t"), T("u", 2), Alu.bitwise_and)
            V.tensor_tensor(T("w"), MV(em), dn(em, 3), Alu.bitwise_and)
            V.tensor_tensor(T("b"), T("w"), T("t", 1), Alu.bitwise_and)
            V.tensor_tensor(T("y"), T("b"), T("b", -1), Alu.bitwise_or)
            V.tensor_tensor(T("q"), T("a"), T("y"), Alu.bitwise_or)
            V.tensor_tensor(T("l2"), T("a"), T("q", -1), Alu.bitwise_or)

            V.tensor_tensor(T("m3"), T("t"), dn(me, 2), Alu.bitwise_and)
            V.tensor_tensor(T("r1"), MV(em), dn(em, 4), Alu.bitwise_and)
            V.tensor_tensor(T("c"), T("r1"), T("m3", 1), Alu.bitwise_and)
            V.tensor_tensor(T("i1"), T("c"), T("c", -1), Alu.bitwise_or)
            V.tensor_tensor(T("l3"), T("c"), T("i1", -1), Alu.bitwise_or)  # row-unshifted

            V.tensor_tensor(
                T("lb"), MV(op, -1),
                lbmC[:, None, :].broadcast_to((P, NB, 8)), Alu.bitwise_or,
            )
            V.tensor_tensor(T("d0"), T("m3"), dn(em, 3), Alu.bitwise_and)
            V.tensor_tensor(T("d1"), T("d0"), dn(nm, 4), Alu.bitwise_and)
            V.tensor_tensor(T("d"), T("d1"), T("lb"), Alu.bitwise_and)
            V.tensor_tensor(T("j1"), T("d"), T("d", -1), Alu.bitwise_or)
            V.tensor_tensor(T("md"), T("j1"), T("d", -2), Alu.bitwise_or)
            V.tensor_tensor(T("o3"), T("m3", 1), MV(nm), Alu.bitwise_and)
            V.tensor_tensor(T("o3"), T("o3"), dn(nm, 4), Alu.bitwise_and)
            V.tensor_tensor(
                T("rb"), MV(op, 5),
                rbmC[:, None, :].broadcast_to((P, NB, 8)), Alu.bitwise_or,
            )
            V.tensor_tensor(T("x"), T("lb"), T("rb"), Alu.bitwise_xor)
            V.tensor_tensor(T("e"), T("o3"), T("x"), Alu.bitwise_and)
            V.tensor_tensor(T("g1"), T("e"), T("e", -1), Alu.bitwise_or)
            V.tensor_tensor(T("g2"), T("e"), T("g1", -1), Alu.bitwise_or)
            V.tensor_tensor(T("r3"), T("md"), T("g2", -1), Alu.bitwise_or)

        line_feats_col()

        # merges into Rg lanes 3..5
        ctp = TMP  # col tiles are "c_*"
        RT = slice(2, 10)
        V.tensor_tensor(Rg[:, 3], lf_row["l2"][:, :, RT],
                        ctp["c_l2"][:, :, RT], Alu.bitwise_or)
        _stt(V, Rg[:, 4], lf_row["l3"][:, :, RT], 1, Alu.bitwise_or,
             ctp["c_l3"][:, :, 1:9])
        V.tensor_tensor(Rg[:, 5], lf_row["r3"][:, :, RT],
                        ctp["c_r3"][:, :, RT], Alu.bitwise_or)

        # ---------- expansion ----------
        OUTCH = {0: (2, 5), 1: (8, 11)}       # persp 0: ch2-4 conn / 8-10 line
        OUTCH_OP = {0: (5, 8), 1: (13, 16)}   # persp 1: ch5-7 / 13-15

        for ck in range(NCHUNK):
            n0 = ck * CB
            outt = cpool.tile([P, CB, 18, 64], DT.float32, name="outt")
            mk = cpool.tile([P, 2, 6, CB, 64], DT.uint32, name="mk", bufs=2)
            dsums = cpool.tile([P, CB, 6], DT.float32, name="dsums", bufs=2)
            dge = cpool.tile([P, CB, 4], DT.float32, name="dge", bufs=2)

            A.activation(outt[:, :, 0, :], myf[:, n0:n0 + CB, :], Act.Copy)
            A.activation(outt[:, :, 1, :], opf[:, n0:n0 + CB, :], Act.Copy)

            for pi in range(2):
                V.tensor_tensor(
                    mk[:, pi].rearrange("p c b (r j) -> p c b r j", j=8),
                    Rg[:, :, n0:n0 + CB, :, None].broadcast_to((P, 6, CB, 8, 8)),
                    masks[:, pi * 64:(pi + 1) * 64]
                    .rearrange("p (r j) -> p r j", j=8)[:, None, None]
                    .broadcast_to((P, 6, CB, 8, 8)),
                    Alu.bitwise_and,
                )
            # conn channels
            V.tensor_scalar(
                outt[:, :, 2:5, :], mk[:, 0, 0:3].rearrange("p c b x -> p b c x"),
                0, None, Alu.not_equal)
            V.tensor_scalar(
                outt[:, :, 5:8, :], mk[:, 1, 0:3].rearrange("p c b x -> p b c x"),
                0, None, Alu.not_equal)
            # line channels
            V.tensor_scalar(
                outt[:, :, 8:11, :], mk[:, 0, 3:6].rearrange("p c b x -> p b c x"),
                0, None, Alu.not_equal)
            V.tensor_scalar(
                outt[:, :, 13:16, :], mk[:, 1, 3:6].rearrange("p c b x -> p b c x"),
                0, None, Alu.not_equal)
            # doubles: per-board cell sums of the merged channels
            V.tensor_reduce(
                dsums[:, :, 0:3], outt[:, :, 8:11, :],
                axis=mybir.AxisListType.X, op=Alu.add)
            V.tensor_reduce(
                dsums[:, :, 3:6], outt[:, :, 13:16, :],
                axis=mybir.AxisListType.X, op=Alu.add)
            s23m = dge[:, :, 1]
            s23o = dge[:, :, 3]
            V.tensor_tensor(s23m, dsums[:, :, 1], dsums[:, :, 2], Alu.add)
            V.tensor_tensor(s23o, dsums[:, :, 4], dsums[:, :, 5], Alu.add)
            V.tensor_scalar(dge[:, :, 0], dsums[:, :, 0], 1.5, None, Alu.is_ge)
            V.tensor_scalar(dge[:, :, 1], s23m, 1.5, None, Alu.is_ge)
            V.tensor_scalar(dge[:, :, 2], dsums[:, :, 3], 1.5, None, Alu.is_ge)
            V.tensor_scalar(dge[:, :, 3], s23o, 1.5, None, Alu.is_ge)
            A.activation(
                outt[:, :, 11:13, :],
                dge[:, :, 0:2, None].broadcast_to((P, CB, 2, 64)), Act.Copy)
            A.activation(
                outt[:, :, 16:18, :],
                dge[:, :, 2:4, None].broadcast_to((P, CB, 2, 64)), Act.Copy)
            nc.sync.dma_start(
                out_v[:, n0:n0 + CB, :], outt.rearrange("p b c x -> p b (c x)"))


_NC_CACHE = None


def _build_nc():
    global _NC_CACHE
    if _NC_CACHE is not None:
        return _NC_CACHE
    nc = bacc.Bacc("TRN2", debug=False, enable_asserts=False)
    state_d = nc.dram_tensor("state", [BPC, 64], DT.float32, kind="ExternalInput").ap()
    side_d = nc.dram_tensor("side", [BPC], DT.float32, kind="ExternalInput").ap()
    out_d = nc.dram_tensor("out", [BPC, 18 * 64], DT.float32, kind="ExternalOutput").ap()
    with tile.TileContext(nc) as tc:
        feature_kernel(tc, out_d, state_d, side_d)
    nc.finalize()
    _NC_CACHE = nc
    return nc


_JIT_CACHE = None


def _get_runner():
    """Build a jitted shard_map runner over the 8 cores, fed with
    pre-sharded jax Arrays (avoids XLA-side resharding programs, which the
    neuron compiler chokes on for these sizes)."""
    global _JIT_CACHE
    if _JIT_CACHE is not None:
        return _JIT_CACHE
    import jax
    from jax.sharding import Mesh, PartitionSpec, NamedSharding
    try:
        from jax.experimental.shard_map import shard_map
    except ImportError:
        from jax.shard_map import shard_map  # newer jax
    from concourse import bass2jax as B2J

    B2J.install_neuronx_cc_hook()
    nc = _build_nc()

    in_names = ["state", "side"]
    out_names = ["out"]
    out_avals = [jax.core.ShapedArray((BPC, 18 * 64), np.float32)]
    all_names = in_names + out_names
    if nc.partition_id_tensor is not None:
        all_names = all_names + [nc.partition_id_tensor.name]

    def _body(state_a, side_a, zeros_a):
        operands = [state_a, side_a, zeros_a]
        if nc.partition_id_tensor is not None:
            operands.append(B2J.partition_id_tensor())
        outs = B2J._bass_exec_p.bind(
            *operands,
            out_avals=tuple(out_avals),
            in_names=tuple(all_names),
            out_names=tuple(out_names),
            lowering_input_output_aliases=(),
            sim_require_finite=True,
            sim_require_nnan=True,
            nc=nc,
        )
        return outs[0]

    devices = jax.devices()[:NCORES]
    mesh = Mesh(np.asarray(devices), ("core",))
    spec = PartitionSpec("core")
    sharded = jax.jit(
        shard_map(
            _body, mesh=mesh,
            in_specs=(spec, spec, spec),
            out_specs=spec,
            check_rep=False,
        ),
        donate_argnums=(2,),
        keep_unused=True,
    )

    def put(shards):
        arrs = [jax.device_put(s, devices[i]) for i, s in enumerate(shards)]
        global_shape = (sum(s.shape[0] for s in shards),) + shards[0].shape[1:]
        return jax.make_array_from_single_device_arrays(
            global_shape, NamedSharding(mesh, spec), arrs
        )

    _JIT_CACHE = (sharded, put)
    return _JIT_CACHE


def kernel(state, side):
    """Full-input entry point: state [32768,8,8] f32, side [32768] f32."""
    state = np.ascontiguousarray(np.asarray(state, dtype=np.float32)).reshape(-1, 64)
    side = np.ascontiguousarray(np.asarray(side, dtype=np.float32)).reshape(-1)
    B = state.shape[0]
    assert B == BPC * NCORES, (B, BPC * NCORES)
    sharded, put = _get_runner()
    state_g = put([state[i * BPC:(i + 1) * BPC] for i in range(NCORES)])
    side_g = put([side[i * BPC:(i + 1) * BPC] for i in range(NCORES)])
    zeros_g = put([np.zeros((BPC, 18 * 64), np.float32) for _ in range(NCORES)])
    out = sharded(state_g, side_g, zeros_g)
    out = np.asarray(out).reshape(NCORES * BPC, 18, 8, 8)
    return out



# revision 7
# speedup vs baseline: 1.0139x; 1.0139x over previous
"""v3 feature kernel: dual-player packed planes (my: bits 0-7, op: bits 16-23),
u8 channel-major output with host-side transpose/cast.

Key changes vs v2:
- Expansion via fused tensor_scalar (byte>>j)&1 on u8 bitcast views of the
  packed planes straight into a u8 channel arena (no mask pass, no compare).
- Doubles channels via OR-fold + nonzero test (marks always come in windows
  of >=2 cells, so count>=2 <=> any bit set).
- Output written channel-major [18, B, 64] as u8; host transposes + casts.
- Scalar engine computes my/op planes via relu(+-state) (side==1 fast path),
  gpsimd does ch0/1 casts and doubles broadcasts.
- Line features reuse conn direction chains (d2/d3 of row and col dirs).
"""
import numpy as np

import concourse.bass as bass
import concourse.bacc as bacc
import concourse.mybir as mybir
import concourse.tile as tile

Alu = mybir.AluOpType
Act = mybir.ActivationFunctionType
DT = mybir.dt

P = 128
NB = 32
NCORES = 8
BPC = P * NB
PAD = 18
R0 = 5
ROWS = slice(R0, R0 + 8)
SEG = 0x00FF00FF

# arena slot -> output channel: [2,3,4, 8,9,10, 11,12, 5,6,7, 13,14,15]
NCH_ARENA = 14


def _stt_raw(eng, out, in0, imm, in1, op0, op1, imm_dt=DT.uint32):
    outs = [eng.lower_ap(out)]
    return eng.add_instruction(
        mybir.InstTensorScalarPtr(
            name=eng.bass.get_next_instruction_name(),
            is_scalar_tensor_tensor=True,
            op0=op0, op1=op1,
            ins=[eng.lower_ap(in0),
                 mybir.ImmediateValue(dtype=imm_dt, value=imm),
                 eng.lower_ap(in1)],
            outs=outs,
        )
    )


def _stt(eng, out, in0, sh, op1, in1):
    if sh > 0:
        _stt_raw(eng, out, in0, sh, in1, Alu.logical_shift_left, op1)
    elif sh < 0:
        _stt_raw(eng, out, in0, -sh, in1, Alu.logical_shift_right, op1)
    else:
        eng.tensor_tensor(out, in0, in1, op1)


def feature_kernel(tc, out_d, state_d, side_d, allones):
    nc = tc.nc
    V, G, A = nc.vector, nc.gpsimd, nc.scalar

    state_v = state_d.rearrange("(p n) c -> p n c", p=P)
    out_v = out_d.rearrange("c (p x) -> p c x", p=P)  # [P, 18, NB*64]

    with (
        tc.tile_pool(name="main", bufs=1) as pool,
        tc.tile_pool(name="inp", bufs=1) as ipool,
    ):
        # ---------- input ----------
        s = ipool.tile([P, NB, 64], DT.float32, name="s")
        nc.sync.dma_start(s[:], state_v)
        myf = ipool.tile([P, NB, 64], DT.float32, name="myf")
        opf = ipool.tile([P, NB, 64], DT.float32, name="opf")
        if allones:
            A.activation(myf[:], s[:], Act.Relu)
            A.activation(opf[:], s[:], Act.Relu, scale=-1.0)
        else:
            side_v = side_d.rearrange("(p n) -> p n", p=P)
            sideT = ipool.tile([P, NB], DT.float32, name="sideT")
            nc.sync.dma_start(sideT[:], side_v)
            sp = ipool.tile([P, NB, 64], DT.float32, name="sp")
            G.tensor_tensor(
                sp[:], s[:], sideT[:, :, None].broadcast_to((P, NB, 64)), Alu.mult
            )
            A.activation(myf[:], sp[:], Act.Relu)
            A.activation(opf[:], sp[:], Act.Relu, scale=-1.0)

        # ch0/ch1 u8 copies (gpsimd)
        chA = pool.tile([P, 2, NB, 64], DT.uint8, name="chA")
        G.tensor_copy(chA[:, 0], myf[:])
        G.tensor_copy(chA[:, 1], opf[:])
        nc.sync.dma_start(out_v[:, 0:2, :],
                          chA.rearrange("p c n x -> p c (n x)"))

        # ---------- guard-zero memsets (vector, overlaps input latency) ----
        PLN = pool.tile([P, 4, NB, PAD], DT.uint32, name="PLN")  # Ad Bd Ed Nd
        V.memset(PLN[:, :, :, 4:5], 0)
        V.memset(PLN[:, :, :, 13:18], 0)
        # padded conn temps: d2c, d3c (dir 1 keepalive), dp2, dp3, dp4, tp3, tp4
        CT = pool.tile([P, 7, NB, PAD], DT.uint32, name="CT")
        V.memset(CT[:, :, :, 4:5], 0)
        V.memset(CT[:, :, :, 13:16], 0)
        # col line-feature arena, 12 rows each, valid rows 2..10
        CNAMES = ["u", "b", "q", "c", "i1", "d", "e", "g1", "g2", "l3",
                  "a", "w", "y", "r1", "lb", "d0", "d1", "j1", "md", "o3",
                  "rb", "x", "l2", "r3"]
        NGUARD = 10  # first 10 names are read at row offsets in col mode
        CIDX = {n: i for i, n in enumerate(CNAMES)}
        LFC = pool.tile([P, len(CNAMES), NB, 12], DT.uint32, name="LFC")
        V.memset(LFC[:, 0:NGUARD, :, 0:2], 0)
        V.memset(LFC[:, 0:NGUARD, :, 10:12], 0)
        lbmC = pool.tile([P, 8], DT.uint32, name="lbmC")
        rbmC = pool.tile([P, 8], DT.uint32, name="rbmC")
        V.memset(lbmC[:], 0)
        V.memset(lbmC[:, 0:1], SEG)
        V.memset(rbmC[:], 0)
        V.memset(rbmC[:, 3:8], SEG)

        # ---------- packing ----------
        myR = pool.tile([P, NB, 8], DT.uint32, name="myR")
        opR = pool.tile([P, NB, 8], DT.uint32, name="opR")
        pk1 = ipool.tile([P, NB * 8, 4], DT.float32, name="pk1")
        pk2 = ipool.tile([P, NB * 8, 2], DT.float32, name="pk2")

        def pack(dst_ap, srcf):
            v = srcf.rearrange("p n (r j2 t) -> p (n r) j2 t", t=2, j2=4)
            a1, b1 = v[:, :, :, 1], v[:, :, :, 0]
            V.scalar_tensor_tensor(pk1[:], a1, 2.0, b1, op0=Alu.mult, op1=Alu.add)
            w2 = pk1.rearrange("p q (k t) -> p q k t", t=2)
            a2, b2 = w2[:, :, :, 1], w2[:, :, :, 0]
            V.scalar_tensor_tensor(pk2[:], a2, 4.0, b2, op0=Alu.mult, op1=Alu.add)
            w3 = pk2.rearrange("p (n r) t -> p n r t", r=8)
            a3, b3 = w3[:, :, :, 1], w3[:, :, :, 0]
            V.scalar_tensor_tensor(dst_ap, a3, 16.0, b3, op0=Alu.mult, op1=Alu.add)

        pack(myR[:], myf)
        pack(opR[:], opf)

        # ---------- planes ----------
        Ad, Bd, Ed, Nd = PLN[:, 0], PLN[:, 1], PLN[:, 2], PLN[:, 3]
        _stt(V, Ad[:, :, ROWS], opR[:], 16, Alu.bitwise_or, myR[:])
        _stt(V, Bd[:, :, ROWS], myR[:], 16, Alu.bitwise_or, opR[:])
        V.tensor_tensor(Ed[:, :, ROWS], Ad[:, :, ROWS], Bd[:, :, ROWS],
                        Alu.bitwise_or)
        V.tensor_scalar(Ed[:, :, ROWS], Ed[:, :, ROWS], SEG, None, Alu.bitwise_xor)
        V.tensor_scalar(Nd[:, :, ROWS], Ad[:, :, ROWS], SEG, None, Alu.bitwise_xor)

        # ---------- connectivity ----------
        # A2/A3/A4: dir-major arenas
        A2 = pool.tile([P, 4, NB, 8], DT.uint32, name="A2")
        A3 = pool.tile([P, 4, NB, 8], DT.uint32, name="A3")
        A4 = pool.tile([P, 4, NB, 8], DT.uint32, name="A4")
        # unpadded temps for dir 0 (bit shifts only)
        d2r = pool.tile([P, NB, 8], DT.uint32, name="d2r")
        d3r = pool.tile([P, NB, 8], DT.uint32, name="d3r")
        u4 = pool.tile([P, NB, 8], DT.uint32, name="u4")
        u5 = pool.tile([P, NB, 8], DT.uint32, name="u5")
        u6 = pool.tile([P, NB, 8], DT.uint32, name="u6")
        mv = Ad[:, :, ROWS]

        # dir 0: (0,1) -- keep d2r/d3r for row line features
        _stt(V, d2r[:], mv, 1, Alu.bitwise_and, mv)
        _stt(V, d3r[:], d2r[:], 1, Alu.bitwise_and, d2r[:])
        _stt(V, u4[:], d3r[:], 1, Alu.bitwise_and, d3r[:])      # d4
        _stt(V, A2[:, 0], d2r[:], -1, Alu.bitwise_or, d2r[:])
        _stt(V, u5[:], d3r[:], -1, Alu.bitwise_or, d3r[:])      # t3
        _stt(V, A3[:, 0], d3r[:], -2, Alu.bitwise_or, u5[:])
        _stt(V, u6[:], u4[:], -1, Alu.bitwise_or, u4[:])        # t4
        _stt(V, A4[:, 0], u6[:], -2, Alu.bitwise_or, u6[:])

        # dir 1: (1,0) -- keep d2c/d3c (padded) for col line features
        d2c, d3c = CT[:, 0], CT[:, 1]
        dp2, dp3, dp4, tp3, tp4 = CT[:, 2], CT[:, 3], CT[:, 4], CT[:, 5], CT[:, 6]

        def fwd(t, di):
            return t[:, :, R0 - di:R0 + 8 - di]

        def bwd(t, di, k=1):
            return t[:, :, R0 + k * di:R0 + 8 + k * di]

        for di_i, (di, dj) in ((1, (1, 0)), (2, (1, 1)), (3, (1, -1))):
            if di_i == 1:
                td2, td3 = d2c, d3c
            else:
                td2, td3 = dp2, dp3
            _stt(V, td2[:, :, ROWS], fwd(Ad, di), dj, Alu.bitwise_and, mv)
            _stt(V, td3[:, :, ROWS], fwd(td2, di), dj, Alu.bitwise_and,
                 td2[:, :, ROWS])
            _stt(V, dp4[:, :, ROWS], fwd(td3, di), dj, Alu.bitwise_and,
                 td3[:, :, ROWS])
            _stt(V, A2[:, di_i], bwd(td2, di), -dj, Alu.bitwise_or,
                 td2[:, :, ROWS])
            _stt(V, tp3[:, :, ROWS], bwd(td3, di), -dj, Alu.bitwise_or,
                 td3[:, :, ROWS])
            _stt(V, A3[:, di_i], bwd(td3, di, 2), -2 * dj, Alu.bitwise_or,
                 tp3[:, :, ROWS])
            _stt(V, tp4[:, :, ROWS], bwd(dp4, di), -dj, Alu.bitwise_or,
                 dp4[:, :, ROWS])
            _stt(V, A4[:, di_i], bwd(tp4, di, 2), -2 * dj, Alu.bitwise_or,
                 tp4[:, :, ROWS])

        # ---------- conn merges: Rg[0..2] ----------
        Rg = pool.tile([P, 6, NB, 8], DT.uint32, name="Rg")
        x2 = pool.tile([P, 4, NB, 8], DT.uint32, name="x2")
        f2 = pool.tile([P, 2, NB, 8], DT.uint32, name="f2")
        # c1 = mv ^ AND(all a2)
        V.tensor_tensor(f2[:], A2[:, 0:2], A2[:, 2:4], Alu.bitwise_and)
        V.tensor_tensor(x2[:, 0], f2[:, 0], f2[:, 1], Alu.bitwise_and)
        V.tensor_tensor(Rg[:, 0], mv, x2[:, 0], Alu.bitwise_xor)
        # c2 = OR(a2^a3)
        V.tensor_tensor(x2[:], A2[:], A3[:], Alu.bitwise_xor)
        V.tensor_tensor(f2[:], x2[:, 0:2], x2[:, 2:4], Alu.bitwise_or)
        V.tensor_tensor(Rg[:, 1], f2[:, 0], f2[:, 1], Alu.bitwise_or)
        # c3 = OR(a3^a4)
        V.tensor_tensor(x2[:], A3[:], A4[:], Alu.bitwise_xor)
        V.tensor_tensor(f2[:], x2[:, 0:2], x2[:, 2:4], Alu.bitwise_or)
        V.tensor_tensor(Rg[:, 2], f2[:, 0], f2[:, 1], Alu.bitwise_or)

        # ---------- channel arena + conn expansion ----------
        arena = pool.tile([P, NCH_ARENA, NB, 64], DT.uint8, name="arena")
        rgb = Rg.bitcast(DT.uint8).rearrange("p c n (r b) -> p c n r b", b=4)

        def expand(slot0, plane0, nplanes, byte, jlist=range(8)):
            """arena[:, slot0:slot0+nplanes, :, r*8+j] =
               (byte(Rg[plane0+c]) >> j) & 1"""
            av = arena.rearrange("p c n (r j) -> p c n r j", j=8)
            for j in jlist:
                V.tensor_scalar(
                    av[:, slot0:slot0 + nplanes, :, :, j],
                    rgb[:, plane0:plane0 + nplanes, :, :, byte],
                    j, 1, op0=Alu.logical_shift_right, op1=Alu.bitwise_and)

        expand(0, 0, 3, 0)   # ch2:5  (conn my)
        nc.sync.dma_start(out_v[:, 2:5, :],
                          arena[:, 0:3].rearrange("p c n x -> p c (n x)"))
        expand(8, 0, 3, 2)   # ch5:8  (conn op)
        nc.sync.dma_start(out_v[:, 5:8, :],
                          arena[:, 8:11].rearrange("p c n x -> p c (n x)"))

        # ---------- line features ----------
        # row mode: bit-shift ops on [P, NB, 8] u32, reusing d2r/d3r
        me, op_, em, nm = mv, Bd[:, :, ROWS], Ed[:, :, ROWS], Nd[:, :, ROWS]
        R = {}

        def rt(n):
            if n not in R:
                R[n] = pool.tile([P, NB, 8], DT.uint32, name="r_" + n)
            return R[n][:]

        # l2 chain: u, w, a'=(u>>1)&d2, b=(d2>>2)&w, y=(b<<1)|b, a=a'>>1,
        #           q=a|y, l2=(q<<1)|a
        _stt(V, rt("u"), em, -1, Alu.bitwise_and, em)
        _stt(V, rt("w"), em, -3, Alu.bitwise_and, em)
        _stt(V, rt("ap"), rt("u"), -1, Alu.bitwise_and, d2r[:])
        _stt(V, rt("b"), d2r[:], -2, Alu.bitwise_and, rt("w"))
        _stt(V, rt("y"), rt("b"), 1, Alu.bitwise_or, rt("b"))
        V.tensor_scalar(rt("a"), rt("ap"), 1, SEG,
                        op0=Alu.logical_shift_right, op1=Alu.bitwise_and)
        V.tensor_tensor(rt("q"), rt("a"), rt("y"), Alu.bitwise_or)
        _stt(V, rt("l2"), rt("q"), 1, Alu.bitwise_or, rt("a"))
        # l3 chain: r1, c=(d3>>3)&r1, i1=(c<<1)|c, l3=(i1<<1)|c (unshifted)
        _stt(V, rt("r1"), em, -4, Alu.bitwise_and, em)
        _stt(V, rt("c"), d3r[:], -3, Alu.bitwise_and, rt("r1"))
        _stt(V, rt("i1"), rt("c"), 1, Alu.bitwise_or, rt("c"))
        _stt(V, rt("l3"), rt("i1"), 1, Alu.bitwise_or, rt("c"))
        # r3 chain
        V.tensor_scalar(rt("lb"), op_, 1, 0x00010001,
                        op0=Alu.logical_shift_left, op1=Alu.bitwise_or)
        _stt(V, rt("d0"), em, -1, Alu.bitwise_and, d3r[:])      # (em>>1)&d3
        _stt(V, rt("d1"), nm, -2, Alu.bitwise_and, rt("d0"))    # (nm>>2)&d0'
        _stt(V, rt("d"), rt("d1"), -2, Alu.bitwise_and, rt("lb"))
        _stt(V, rt("j1"), rt("d"), 1, Alu.bitwise_or, rt("d"))
        _stt(V, rt("md"), rt("d"), 2, Alu.bitwise_or, rt("j1"))
        _stt(V, rt("o3"), d3r[:], -3, Alu.bitwise_and, nm)
        _stt(V, rt("o3"), nm, -4, Alu.bitwise_and, rt("o3"))
        V.tensor_scalar(rt("rb"), op_, 5, 0x00F800F8,
                        op0=Alu.logical_shift_right, op1=Alu.bitwise_or)
        V.tensor_tensor(rt("x"), rt("lb"), rt("rb"), Alu.bitwise_xor)
        V.tensor_tensor(rt("e"), rt("o3"), rt("x"), Alu.bitwise_and)
        _stt(V, rt("g1"), rt("e"), 1, Alu.bitwise_or, rt("e"))
        _stt(V, rt("g2"), rt("g1"), 1, Alu.bitwise_or, rt("e"))
        _stt(V, rt("r3"), rt("g2"), 1, Alu.bitwise_or, rt("md"))

        # col mode: row-offset TT ops; t = dn(d2c,1), m3 = dn(d3c,2)
        R0T = 2

        def T(n, k=0):
            t = LFC[:, CIDX[n]]
            return t[:, :, R0T + k:R0T + 8 + k]

        def dn(x, k):
            return x[:, :, R0 + k:R0 + 8 + k]

        def MV(x, k=0):
            return x[:, :, R0 + k:R0 + 8 + k]

        tcol = dn(d2c, 1)     # me[r]&me[r+1]
        m3col_p1 = dn(d3c, 3)  # m3 at +1 row = d3c[r+3]
        V.tensor_tensor(T("u"), MV(Ed), dn(Ed, 1), Alu.bitwise_and)
        V.tensor_tensor(T("a"), tcol, T("u", 2), Alu.bitwise_and)
        V.tensor_tensor(T("w"), MV(Ed), dn(Ed, 3), Alu.bitwise_and)
        V.tensor_tensor(T("b"), T("w"), dn(d2c, 2), Alu.bitwise_and)  # w & t(+1)
        V.tensor_tensor(T("y"), T("b"), T("b", -1), Alu.bitwise_or)
        V.tensor_tensor(T("q"), T("a"), T("y"), Alu.bitwise_or)
        V.tensor_tensor(T("l2"), T("a"), T("q", -1), Alu.bitwise_or)

        V.tensor_tensor(T("r1"), MV(Ed), dn(Ed, 4), Alu.bitwise_and)
        V.tensor_tensor(T("c"), T("r1"), m3col_p1, Alu.bitwise_and)
        V.tensor_tensor(T("i1"), T("c"), T("c", -1), Alu.bitwise_or)
        V.tensor_tensor(T("l3"), T("c"), T("i1", -1), Alu.bitwise_or)

        V.tensor_tensor(
            T("lb"), MV(Bd, -1),
            lbmC[:, None, :].broadcast_to((P, NB, 8)), Alu.bitwise_or)
        V.tensor_tensor(T("d0"), dn(d3c, 2), dn(Ed, 3), Alu.bitwise_and)
        V.tensor_tensor(T("d1"), T("d0"), dn(Nd, 4), Alu.bitwise_and)
        V.tensor_tensor(T("d"), T("d1"), T("lb"), Alu.bitwise_and)
        V.tensor_tensor(T("j1"), T("d"), T("d", -1), Alu.bitwise_or)
        V.tensor_tensor(T("md"), T("j1"), T("d", -2), Alu.bitwise_or)
        V.tensor_tensor(T("o3"), m3col_p1, MV(Nd), Alu.bitwise_and)
        V.tensor_tensor(T("o3"), T("o3"), dn(Nd, 4), Alu.bitwise_and)
        V.tensor_tensor(
            T("rb"), MV(Bd, 5),
            rbmC[:, None, :].broadcast_to((P, NB, 8)), Alu.bitwise_or)
        V.tensor_tensor(T("x"), T("lb"), T("rb"), Alu.bitwise_xor)
        V.tensor_tensor(T("e"), T("o3"), T("x"), Alu.bitwise_and)
        V.tensor_tensor(T("g1"), T("e"), T("e", -1), Alu.bitwise_or)
        V.tensor_tensor(T("g2"), T("e"), T("g1", -1), Alu.bitwise_or)
        V.tensor_tensor(T("r3"), T("md"), T("g2", -1), Alu.bitwise_or)

        # merges into Rg[3..5]
        V.tensor_tensor(Rg[:, 3], rt("l2"), T("l2"), Alu.bitwise_or)
        _stt(V, Rg[:, 4], rt("l3"), 1, Alu.bitwise_or, T("l3", -1))
        V.tensor_tensor(Rg[:, 5], rt("r3"), T("r3"), Alu.bitwise_or)

        # ---------- doubles ----------
        orf2 = pool.tile([P, NB], DT.uint32, name="orf2")
        orf3 = pool.tile([P, NB], DT.uint32, name="orf3")
        lr = pool.tile([P, NB, 8], DT.uint32, name="lr")
        V.tensor_reduce(orf2[:], Rg[:, 3], axis=mybir.AxisListType.X,
                        op=Alu.bitwise_or)
        V.tensor_tensor(lr[:], Rg[:, 4], Rg[:, 5], Alu.bitwise_or)
        V.tensor_reduce(orf3[:], lr[:], axis=mybir.AxisListType.X,
                        op=Alu.bitwise_or)
        dge = pool.tile([P, 4, NB], DT.uint8, name="dge")
        o2b = orf2.bitcast(DT.uint8)
        o3b = orf3.bitcast(DT.uint8)
        V.tensor_scalar(dge[:, 0], o2b[:, 0::4], 0, None, Alu.not_equal)  # ch11
        V.tensor_scalar(dge[:, 1], o3b[:, 0::4], 0, None, Alu.not_equal)  # ch12
        V.tensor_scalar(dge[:, 2], o2b[:, 2::4], 0, None, Alu.not_equal)  # ch16
        V.tensor_scalar(dge[:, 3], o3b[:, 2::4], 0, None, Alu.not_equal)  # ch17

        # ---------- line expansion + doubles broadcast ----------
        expand(3, 3, 3, 0)   # ch8:11 (line my)
        nc.sync.dma_start(out_v[:, 8:11, :],
                          arena[:, 3:6].rearrange("p c n x -> p c (n x)"))
        G.tensor_copy(arena[:, 6], dge[:, 0, :, None].broadcast_to((P, NB, 64)))
        G.tensor_copy(arena[:, 7], dge[:, 1, :, None].broadcast_to((P, NB, 64)))
        nc.sync.dma_start(out_v[:, 11:13, :],
                          arena[:, 6:8].rearrange("p c n x -> p c (n x)"))
        expand(11, 3, 3, 2)  # ch13:16 (line op)
        nc.sync.dma_start(out_v[:, 13:16, :],
                          arena[:, 11:14].rearrange("p c n x -> p c (n x)"))
        chD = pool.tile([P, 2, NB, 64], DT.uint8, name="chD")
        G.tensor_copy(chD[:, 0], dge[:, 2, :, None].broadcast_to((P, NB, 64)))
        G.tensor_copy(chD[:, 1], dge[:, 3, :, None].broadcast_to((P, NB, 64)))
        nc.sync.dma_start(out_v[:, 16:18, :],
                          chD.rearrange("p c n x -> p c (n x)"))


_NC_CACHE = {}


def _build_nc(allones):
    if allones in _NC_CACHE:
        return _NC_CACHE[allones]
    nc = bacc.Bacc("TRN2", debug=False, enable_asserts=False)
    state_d = nc.dram_tensor("state", [BPC, 64], DT.float32, kind="ExternalInput").ap()
    side_d = nc.dram_tensor("side", [BPC], DT.float32, kind="ExternalInput").ap()
    out_d = nc.dram_tensor("out", [18, BPC * 64], DT.uint8, kind="ExternalOutput").ap()
    with tile.TileContext(nc) as tc:
        feature_kernel(tc, out_d, state_d, side_d, allones)
    nc.finalize()
    _NC_CACHE[allones] = nc
    return nc


_JIT_CACHE = {}


def _get_runner(allones):
    if allones in _JIT_CACHE:
        return _JIT_CACHE[allones]
    import jax
    from jax.sharding import Mesh, PartitionSpec, NamedSharding
    try:
        from jax.experimental.shard_map import shard_map
    except ImportError:
        from jax.shard_map import shard_map  # newer jax
    from concourse import bass2jax as B2J

    B2J.install_neuronx_cc_hook()
    nc = _build_nc(allones)

    in_names = ["state", "side"]
    out_names = ["out"]
    out_avals = [jax.core.ShapedArray((18, BPC * 64), np.uint8)]
    all_names = in_names + out_names
    if nc.partition_id_tensor is not None:
        all_names = all_names + [nc.partition_id_tensor.name]

    def _body(state_a, side_a, zeros_a):
        operands = [state_a, side_a, zeros_a]
        if nc.partition_id_tensor is not None:
            operands.append(B2J.partition_id_tensor())
        outs = B2J._bass_exec_p.bind(
            *operands,
            out_avals=tuple(out_avals),
            in_names=tuple(all_names),
            out_names=tuple(out_names),
            lowering_input_output_aliases=(),
            sim_require_finite=True,
            sim_require_nnan=True,
            nc=nc,
        )
        return outs[0]

    devices = jax.devices()[:NCORES]
    mesh = Mesh(np.asarray(devices), ("core",))
    spec = PartitionSpec("core")
    sharded = jax.jit(
        shard_map(
            _body, mesh=mesh,
            in_specs=(spec, spec, spec),
            out_specs=spec,
            check_rep=False,
        ),
        donate_argnums=(2,),
        keep_unused=True,
    )

    def put(shards):
        arrs = [jax.device_put(s, devices[i]) for i, s in enumerate(shards)]
        global_shape = (sum(s.shape[0] for s in shards),) + shards[0].shape[1:]
        return jax.make_array_from_single_device_arrays(
            global_shape, NamedSharding(mesh, spec), arrs
        )

    _JIT_CACHE[allones] = (sharded, put)
    return _JIT_CACHE[allones]


def kernel(state, side):
    """Full-input entry point: state [32768,8,8] f32, side [32768] f32."""
    state = np.ascontiguousarray(np.asarray(state, dtype=np.float32)).reshape(-1, 64)
    side = np.ascontiguousarray(np.asarray(side, dtype=np.float32)).reshape(-1)
    B = state.shape[0]
    assert B == BPC * NCORES, (B, BPC * NCORES)
    allones = bool(np.all(side == 1.0))
    sharded, put = _get_runner(allones)
    state_g = put([state[i * BPC:(i + 1) * BPC] for i in range(NCORES)])
    side_g = put([side[i * BPC:(i + 1) * BPC] for i in range(NCORES)])
    zeros_g = put([np.zeros((18, BPC * 64), np.uint8) for _ in range(NCORES)])
    out = sharded(state_g, side_g, zeros_g)
    out = np.asarray(out).reshape(NCORES, 18, BPC, 64)
    out = out.transpose(0, 2, 1, 3).reshape(NCORES * BPC, 18, 8, 8)
    return out.astype(np.float32)


# revision 11
# speedup vs baseline: 1.5791x; 1.5575x over previous
"""v3 feature kernel: dual-player packed planes (my: bits 0-7, op: bits 16-23),
u8 channel-major output with host-side transpose/cast.

Key changes vs v2:
- Expansion via fused tensor_scalar (byte>>j)&1 on u8 bitcast views of the
  packed planes straight into a u8 channel arena (no mask pass, no compare).
- Doubles channels via OR-fold + nonzero test (marks always come in windows
  of >=2 cells, so count>=2 <=> any bit set).
- Output written channel-major [18, B, 64] as u8; host transposes + casts.
- Scalar engine computes my/op planes via relu(+-state) (side==1 fast path),
  gpsimd does ch0/1 casts and doubles broadcasts.
- Line features reuse conn direction chains (d2/d3 of row and col dirs).
"""
import numpy as np

import concourse.bass as bass
import concourse.bacc as bacc
import concourse.mybir as mybir
import concourse.tile as tile

Alu = mybir.AluOpType
Act = mybir.ActivationFunctionType
DT = mybir.dt

P = 128
NB = 32
NCORES = 8
BPC = P * NB
PAD = 18
R0 = 5
ROWS = slice(R0, R0 + 8)
SEG = 0x00FF00FF

# arena slot -> output channel: [2,3,4, 8,9,10, 11,12, 5,6,7, 13,14,15]
NCH_ARENA = 14


def _stt_raw(eng, out, in0, imm, in1, op0, op1, imm_dt=DT.uint32):
    outs = [eng.lower_ap(out)]
    return eng.add_instruction(
        mybir.InstTensorScalarPtr(
            name=eng.bass.get_next_instruction_name(),
            is_scalar_tensor_tensor=True,
            op0=op0, op1=op1,
            ins=[eng.lower_ap(in0),
                 mybir.ImmediateValue(dtype=imm_dt, value=imm),
                 eng.lower_ap(in1)],
            outs=outs,
        )
    )


def _stt(eng, out, in0, sh, op1, in1):
    if sh > 0:
        _stt_raw(eng, out, in0, sh, in1, Alu.logical_shift_left, op1)
    elif sh < 0:
        _stt_raw(eng, out, in0, -sh, in1, Alu.logical_shift_right, op1)
    else:
        eng.tensor_tensor(out, in0, in1, op1)


def feature_kernel(tc, out_d, state_d, side_d, allones):
    nc = tc.nc
    V, G, A = nc.vector, nc.gpsimd, nc.scalar

    state_v = state_d.rearrange("(p n) c -> p n c", p=P)
    out_v = out_d.rearrange("c (p x) -> p c x", p=P)  # [P, 18, NB*64]

    with (
        tc.tile_pool(name="main", bufs=1) as pool,
        tc.tile_pool(name="inp", bufs=1) as ipool,
    ):
        # ---------- input ----------
        s = ipool.tile([P, NB, 64], DT.float32, name="s")
        nc.sync.dma_start(s[:], state_v)
        myf = ipool.tile([P, NB, 64], DT.float32, name="myf")
        opf = ipool.tile([P, NB, 64], DT.float32, name="opf")
        if allones:
            A.activation(myf[:], s[:], Act.Relu)
            A.activation(opf[:], s[:], Act.Relu, scale=-1.0)
        else:
            side_v = side_d.rearrange("(p n) -> p n", p=P)
            sideT = ipool.tile([P, NB], DT.float32, name="sideT")
            nc.sync.dma_start(sideT[:], side_v)
            sp = ipool.tile([P, NB, 64], DT.float32, name="sp")
            G.tensor_tensor(
                sp[:], s[:], sideT[:, :, None].broadcast_to((P, NB, 64)), Alu.mult
            )
            A.activation(myf[:], sp[:], Act.Relu)
            A.activation(opf[:], sp[:], Act.Relu, scale=-1.0)

        # ch0/ch1 u8 copies (scalar engine), j-major cell order
        chA = pool.tile([P, 2, NB, 64], DT.uint8, name="chA")
        A.activation(chA[:, 0].rearrange("p n (j r) -> p n j r", r=8),
                     myf.rearrange("p n (r j) -> p n j r", j=8), Act.Copy)
        A.activation(chA[:, 1].rearrange("p n (j r) -> p n j r", r=8),
                     opf.rearrange("p n (r j) -> p n j r", j=8), Act.Copy)
        nc.sync.dma_start(out_v[:, 0:2, :],
                          chA.rearrange("p c n x -> p c (n x)"))

        # ---------- guard-zero memsets (vector, overlaps input latency) ----
        PLN = pool.tile([P, 4, NB, PAD], DT.uint32, name="PLN")  # Ad Bd Ed Nd
        V.memset(PLN[:, :, :, 4:5], 0)
        V.memset(PLN[:, :, :, 13:18], 0)
        # padded conn temps: d2c, d3c (dir 1 keepalive), dp2, dp3, dp4, tp3, tp4
        CT = pool.tile([P, 7, NB, PAD], DT.uint32, name="CT")
        V.memset(CT[:, :, :, 4:5], 0)
        V.memset(CT[:, :, :, 13:16], 0)
        # col line-feature arena, 12 rows each, valid rows 2..10
        CNAMES = ["u", "b", "q", "c", "i1", "d", "e", "g1", "g2", "l3",
                  "a", "w", "y", "r1", "lb", "d0", "d1", "j1", "md", "o3",
                  "rb", "x", "l2", "r3"]
        NGUARD = 10  # first 10 names are read at row offsets in col mode
        CIDX = {n: i for i, n in enumerate(CNAMES)}
        LFC = pool.tile([P, len(CNAMES), NB, 12], DT.uint32, name="LFC")
        V.memset(LFC[:, 0:NGUARD, :, 0:2], 0)
        V.memset(LFC[:, 0:NGUARD, :, 10:12], 0)
        lbmC = pool.tile([P, 8], DT.uint32, name="lbmC")
        rbmC = pool.tile([P, 8], DT.uint32, name="rbmC")
        V.memset(lbmC[:], 0)
        V.memset(lbmC[:, 0:1], SEG)
        V.memset(rbmC[:], 0)
        V.memset(rbmC[:, 3:8], SEG)

        # ---------- packing ----------
        myR = pool.tile([P, NB, 8], DT.uint32, name="myR")
        opR = pool.tile([P, NB, 8], DT.uint32, name="opR")
        pk1 = ipool.tile([P, NB * 8, 4], DT.float32, name="pk1")
        pk2 = ipool.tile([P, NB * 8, 2], DT.float32, name="pk2")

        def pack(dst_ap, srcf):
            v = srcf.rearrange("p n (r j2 t) -> p (n r) j2 t", t=2, j2=4)
            a1, b1 = v[:, :, :, 1], v[:, :, :, 0]
            V.scalar_tensor_tensor(pk1[:], a1, 2.0, b1, op0=Alu.mult, op1=Alu.add)
            w2 = pk1.rearrange("p q (k t) -> p q k t", t=2)
            a2, b2 = w2[:, :, :, 1], w2[:, :, :, 0]
            V.scalar_tensor_tensor(pk2[:], a2, 4.0, b2, op0=Alu.mult, op1=Alu.add)
            w3 = pk2.rearrange("p (n r) t -> p n r t", r=8)
            a3, b3 = w3[:, :, :, 1], w3[:, :, :, 0]
            V.scalar_tensor_tensor(dst_ap, a3, 16.0, b3, op0=Alu.mult, op1=Alu.add)

        pack(myR[:], myf)
        pack(opR[:], opf)

        # ---------- planes ----------
        Ad, Bd, Ed, Nd = PLN[:, 0], PLN[:, 1], PLN[:, 2], PLN[:, 3]
        _stt(V, Ad[:, :, ROWS], opR[:], 16, Alu.bitwise_or, myR[:])
        _stt(V, Bd[:, :, ROWS], myR[:], 16, Alu.bitwise_or, opR[:])
        V.tensor_tensor(Ed[:, :, ROWS], Ad[:, :, ROWS], Bd[:, :, ROWS],
                        Alu.bitwise_or)
        V.tensor_scalar(Ed[:, :, ROWS], Ed[:, :, ROWS], SEG, None, Alu.bitwise_xor)
        V.tensor_scalar(Nd[:, :, ROWS], Ad[:, :, ROWS], SEG, None, Alu.bitwise_xor)

        # ---------- connectivity ----------
        # A2/A3/A4: dir-major arenas
        A2 = pool.tile([P, 4, NB, 8], DT.uint32, name="A2")
        A3 = pool.tile([P, 4, NB, 8], DT.uint32, name="A3")
        A4 = pool.tile([P, 4, NB, 8], DT.uint32, name="A4")
        # unpadded temps for dir 0 (bit shifts only)
        d2r = pool.tile([P, NB, 8], DT.uint32, name="d2r")
        d3r = pool.tile([P, NB, 8], DT.uint32, name="d3r")
        u4 = pool.tile([P, NB, 8], DT.uint32, name="u4")
        u5 = pool.tile([P, NB, 8], DT.uint32, name="u5")
        u6 = pool.tile([P, NB, 8], DT.uint32, name="u6")
        mv = Ad[:, :, ROWS]

        # dir 0: (0,1) -- keep d2r/d3r for row line features
        _stt(V, d2r[:], mv, 1, Alu.bitwise_and, mv)
        _stt(V, d3r[:], d2r[:], 1, Alu.bitwise_and, d2r[:])
        _stt(V, u4[:], d3r[:], 1, Alu.bitwise_and, d3r[:])      # d4
        _stt(V, A2[:, 0], d2r[:], -1, Alu.bitwise_or, d2r[:])
        _stt(V, u5[:], d3r[:], -1, Alu.bitwise_or, d3r[:])      # t3
        _stt(V, A3[:, 0], d3r[:], -2, Alu.bitwise_or, u5[:])
        _stt(V, u6[:], u4[:], -1, Alu.bitwise_or, u4[:])        # t4
        _stt(V, A4[:, 0], u6[:], -2, Alu.bitwise_or, u6[:])

        # dir 1: (1,0) -- keep d2c/d3c (padded) for col line features
        d2c, d3c = CT[:, 0], CT[:, 1]
        dp2, dp3, dp4, tp3, tp4 = CT[:, 2], CT[:, 3], CT[:, 4], CT[:, 5], CT[:, 6]

        def fwd(t, di):
            return t[:, :, R0 - di:R0 + 8 - di]

        def bwd(t, di, k=1):
            return t[:, :, R0 + k * di:R0 + 8 + k * di]

        for di_i, (di, dj) in ((1, (1, 0)), (2, (1, 1)), (3, (1, -1))):
            if di_i == 1:
                td2, td3 = d2c, d3c
            else:
                td2, td3 = dp2, dp3
            _stt(V, td2[:, :, ROWS], fwd(Ad, di), dj, Alu.bitwise_and, mv)
            _stt(V, td3[:, :, ROWS], fwd(td2, di), dj, Alu.bitwise_and,
                 td2[:, :, ROWS])
            _stt(V, dp4[:, :, ROWS], fwd(td3, di), dj, Alu.bitwise_and,
                 td3[:, :, ROWS])
            _stt(V, A2[:, di_i], bwd(td2, di), -dj, Alu.bitwise_or,
                 td2[:, :, ROWS])
            _stt(V, tp3[:, :, ROWS], bwd(td3, di), -dj, Alu.bitwise_or,
                 td3[:, :, ROWS])
            _stt(V, A3[:, di_i], bwd(td3, di, 2), -2 * dj, Alu.bitwise_or,
                 tp3[:, :, ROWS])
            _stt(V, tp4[:, :, ROWS], bwd(dp4, di), -dj, Alu.bitwise_or,
                 dp4[:, :, ROWS])
            _stt(V, A4[:, di_i], bwd(tp4, di, 2), -2 * dj, Alu.bitwise_or,
                 tp4[:, :, ROWS])

        # ---------- conn merges: Rg[0..2] ----------
        Rg = pool.tile([P, 6, NB, 8], DT.uint32, name="Rg")
        x2 = pool.tile([P, 4, NB, 8], DT.uint32, name="x2")
        f2 = pool.tile([P, 2, NB, 8], DT.uint32, name="f2")
        # c1 = mv ^ AND(all a2)
        V.tensor_tensor(f2[:], A2[:, 0:2], A2[:, 2:4], Alu.bitwise_and)
        V.tensor_tensor(x2[:, 0], f2[:, 0], f2[:, 1], Alu.bitwise_and)
        V.tensor_tensor(Rg[:, 0], mv, x2[:, 0], Alu.bitwise_xor)
        # c2 = OR(a2^a3)
        V.tensor_tensor(x2[:], A2[:], A3[:], Alu.bitwise_xor)
        V.tensor_tensor(f2[:], x2[:, 0:2], x2[:, 2:4], Alu.bitwise_or)
        V.tensor_tensor(Rg[:, 1], f2[:, 0], f2[:, 1], Alu.bitwise_or)
        # c3 = OR(a3^a4)
        V.tensor_tensor(x2[:], A3[:], A4[:], Alu.bitwise_xor)
        V.tensor_tensor(f2[:], x2[:, 0:2], x2[:, 2:4], Alu.bitwise_or)
        V.tensor_tensor(Rg[:, 2], f2[:, 0], f2[:, 1], Alu.bitwise_or)

        # ---------- channel arena + conn expansion ----------
        # arena cell order is j-major: [ch, board, j, r]; host swaps j/r.
        arena = pool.tile([P, NCH_ARENA, NB, 64], DT.uint8, name="arena")
        rgb = Rg.bitcast(DT.uint8).rearrange("p c n (r b) -> p c n r b", b=4)
        # unit-stride byte planes: [persp, plane, board, row-word]
        RB = pool.tile([P, 2, 6, NB, 8], DT.uint8, name="RB")

        def compact(persp, plane0, nplanes):
            A.activation(RB[:, persp, plane0:plane0 + nplanes],
                         rgb[:, plane0:plane0 + nplanes, :, :, 2 * persp],
                         Act.Copy)

        def expand(slot0, plane0, nplanes, persp, jlist=range(8)):
            """arena[:, slot0+c, :, j*8+r] = (RB[persp, plane0+c] >> j) & 1"""
            av = arena.rearrange("p c n (j r) -> p c n j r", r=8)
            for j in jlist:
                V.tensor_scalar(
                    av[:, slot0:slot0 + nplanes, :, j, :],
                    RB[:, persp, plane0:plane0 + nplanes],
                    j, 1, op0=Alu.logical_shift_right, op1=Alu.bitwise_and)

        compact(0, 0, 3)
        compact(1, 0, 3)
        expand(0, 0, 3, 0)   # ch2:5  (conn my)
        nc.sync.dma_start(out_v[:, 2:5, :],
                          arena[:, 0:3].rearrange("p c n x -> p c (n x)"))
        expand(8, 0, 3, 1)   # ch5:8  (conn op)
        nc.sync.dma_start(out_v[:, 5:8, :],
                          arena[:, 8:11].rearrange("p c n x -> p c (n x)"))

        # ---------- line features ----------
        # row mode: bit-shift ops on [P, NB, 8] u32, reusing d2r/d3r
        me, op_, em, nm = mv, Bd[:, :, ROWS], Ed[:, :, ROWS], Nd[:, :, ROWS]
        R = {}

        def rt(n):
            if n not in R:
                R[n] = pool.tile([P, NB, 8], DT.uint32, name="r_" + n)
            return R[n][:]

        # l2 chain: u, w, a'=(u>>1)&d2, b=(d2>>2)&w, y=(b<<1)|b, a=a'>>1,
        #           q=a|y, l2=(q<<1)|a
        _stt(V, rt("u"), em, -1, Alu.bitwise_and, em)
        _stt(V, rt("w"), em, -3, Alu.bitwise_and, em)
        _stt(V, rt("ap"), rt("u"), -1, Alu.bitwise_and, d2r[:])
        _stt(V, rt("b"), d2r[:], -2, Alu.bitwise_and, rt("w"))
        _stt(V, rt("y"), rt("b"), 1, Alu.bitwise_or, rt("b"))
        V.tensor_scalar(rt("a"), rt("ap"), 1, SEG,
                        op0=Alu.logical_shift_right, op1=Alu.bitwise_and)
        V.tensor_tensor(rt("q"), rt("a"), rt("y"), Alu.bitwise_or)
        _stt(V, rt("l2"), rt("q"), 1, Alu.bitwise_or, rt("a"))
        # l3 chain: r1, c=(d3>>3)&r1, i1=(c<<1)|c, l3=(i1<<1)|c (unshifted)
        _stt(V, rt("r1"), em, -4, Alu.bitwise_and, em)
        _stt(V, rt("c"), d3r[:], -3, Alu.bitwise_and, rt("r1"))
        _stt(V, rt("i1"), rt("c"), 1, Alu.bitwise_or, rt("c"))
        _stt(V, rt("l3"), rt("i1"), 1, Alu.bitwise_or, rt("c"))
        # r3 chain
        V.tensor_scalar(rt("lb"), op_, 1, 0x00010001,
                        op0=Alu.logical_shift_left, op1=Alu.bitwise_or)
        _stt(V, rt("d0"), em, -1, Alu.bitwise_and, d3r[:])      # (em>>1)&d3
        _stt(V, rt("d1"), nm, -2, Alu.bitwise_and, rt("d0"))    # (nm>>2)&d0'
        _stt(V, rt("d"), rt("d1"), -2, Alu.bitwise_and, rt("lb"))
        _stt(V, rt("j1"), rt("d"), 1, Alu.bitwise_or, rt("d"))
        _stt(V, rt("md"), rt("d"), 2, Alu.bitwise_or, rt("j1"))
        _stt(V, rt("o3"), d3r[:], -3, Alu.bitwise_and, nm)
        _stt(V, rt("o3"), nm, -4, Alu.bitwise_and, rt("o3"))
        V.tensor_scalar(rt("rb"), op_, 5, 0x00F800F8,
                        op0=Alu.logical_shift_right, op1=Alu.bitwise_or)
        V.tensor_tensor(rt("x"), rt("lb"), rt("rb"), Alu.bitwise_xor)
        V.tensor_tensor(rt("e"), rt("o3"), rt("x"), Alu.bitwise_and)
        _stt(V, rt("g1"), rt("e"), 1, Alu.bitwise_or, rt("e"))
        _stt(V, rt("g2"), rt("g1"), 1, Alu.bitwise_or, rt("e"))
        _stt(V, rt("r3"), rt("g2"), 1, Alu.bitwise_or, rt("md"))

        # col mode: row-offset TT ops; t = dn(d2c,1), m3 = dn(d3c,2)
        R0T = 2

        def T(n, k=0):
            t = LFC[:, CIDX[n]]
            return t[:, :, R0T + k:R0T + 8 + k]

        def dn(x, k):
            return x[:, :, R0 + k:R0 + 8 + k]

        def MV(x, k=0):
            return x[:, :, R0 + k:R0 + 8 + k]

        tcol = dn(d2c, 1)     # me[r]&me[r+1]
        m3col_p1 = dn(d3c, 3)  # m3 at +1 row = d3c[r+3]
        V.tensor_tensor(T("u"), MV(Ed), dn(Ed, 1), Alu.bitwise_and)
        V.tensor_tensor(T("a"), tcol, T("u", 2), Alu.bitwise_and)
        V.tensor_tensor(T("w"), MV(Ed), dn(Ed, 3), Alu.bitwise_and)
        V.tensor_tensor(T("b"), T("w"), dn(d2c, 2), Alu.bitwise_and)  # w & t(+1)
        V.tensor_tensor(T("y"), T("b"), T("b", -1), Alu.bitwise_or)
        V.tensor_tensor(T("q"), T("a"), T("y"), Alu.bitwise_or)
        V.tensor_tensor(T("l2"), T("a"), T("q", -1), Alu.bitwise_or)

        V.tensor_tensor(T("r1"), MV(Ed), dn(Ed, 4), Alu.bitwise_and)
        V.tensor_tensor(T("c"), T("r1"), m3col_p1, Alu.bitwise_and)
        V.tensor_tensor(T("i1"), T("c"), T("c", -1), Alu.bitwise_or)
        V.tensor_tensor(T("l3"), T("c"), T("i1", -1), Alu.bitwise_or)

        V.tensor_tensor(
            T("lb"), MV(Bd, -1),
            lbmC[:, None, :].broadcast_to((P, NB, 8)), Alu.bitwise_or)
        V.tensor_tensor(T("d0"), dn(d3c, 2), dn(Ed, 3), Alu.bitwise_and)
        V.tensor_tensor(T("d1"), T("d0"), dn(Nd, 4), Alu.bitwise_and)
        V.tensor_tensor(T("d"), T("d1"), T("lb"), Alu.bitwise_and)
        V.tensor_tensor(T("j1"), T("d"), T("d", -1), Alu.bitwise_or)
        V.tensor_tensor(T("md"), T("j1"), T("d", -2), Alu.bitwise_or)
        V.tensor_tensor(T("o3"), m3col_p1, MV(Nd), Alu.bitwise_and)
        V.tensor_tensor(T("o3"), T("o3"), dn(Nd, 4), Alu.bitwise_and)
        V.tensor_tensor(
            T("rb"), MV(Bd, 5),
            rbmC[:, None, :].broadcast_to((P, NB, 8)), Alu.bitwise_or)
        V.tensor_tensor(T("x"), T("lb"), T("rb"), Alu.bitwise_xor)
        V.tensor_tensor(T("e"), T("o3"), T("x"), Alu.bitwise_and)
        V.tensor_tensor(T("g1"), T("e"), T("e", -1), Alu.bitwise_or)
        V.tensor_tensor(T("g2"), T("e"), T("g1", -1), Alu.bitwise_or)
        V.tensor_tensor(T("r3"), T("md"), T("g2", -1), Alu.bitwise_or)

        # merges into Rg[3..5]
        V.tensor_tensor(Rg[:, 3], rt("l2"), T("l2"), Alu.bitwise_or)
        _stt(V, Rg[:, 4], rt("l3"), 1, Alu.bitwise_or, T("l3", -1))
        V.tensor_tensor(Rg[:, 5], rt("r3"), T("r3"), Alu.bitwise_or)

        # ---------- doubles ----------
        orf2 = pool.tile([P, NB], DT.uint32, name="orf2")
        orf3 = pool.tile([P, NB], DT.uint32, name="orf3")
        lr = pool.tile([P, NB, 8], DT.uint32, name="lr")
        V.tensor_reduce(orf2[:], Rg[:, 3], axis=mybir.AxisListType.X,
                        op=Alu.bitwise_or)
        V.tensor_tensor(lr[:], Rg[:, 4], Rg[:, 5], Alu.bitwise_or)
        V.tensor_reduce(orf3[:], lr[:], axis=mybir.AxisListType.X,
                        op=Alu.bitwise_or)
        dgef = pool.tile([P, 4, NB], DT.float32, name="dgef")
        o2b = orf2.bitcast(DT.uint8)
        o3b = orf3.bitcast(DT.uint8)
        V.tensor_scalar(dgef[:, 0], o2b[:, 0::4], 0, None, Alu.not_equal)  # ch11
        V.tensor_scalar(dgef[:, 1], o3b[:, 0::4], 0, None, Alu.not_equal)  # ch12
        V.tensor_scalar(dgef[:, 2], o2b[:, 2::4], 0, None, Alu.not_equal)  # ch16
        V.tensor_scalar(dgef[:, 3], o3b[:, 2::4], 0, None, Alu.not_equal)  # ch17

        # ---------- line expansion + doubles broadcast ----------
        compact(0, 3, 3)
        compact(1, 3, 3)
        expand(3, 3, 3, 0)   # ch8:11 (line my)
        nc.sync.dma_start(out_v[:, 8:11, :],
                          arena[:, 3:6].rearrange("p c n x -> p c (n x)"))
        A.activation(arena[:, 6],
                     dgef[:, 0, :, None].broadcast_to((P, NB, 64)), Act.Copy)
        A.activation(arena[:, 7],
                     dgef[:, 1, :, None].broadcast_to((P, NB, 64)), Act.Copy)
        nc.sync.dma_start(out_v[:, 11:13, :],
                          arena[:, 6:8].rearrange("p c n x -> p c (n x)"))
        expand(11, 3, 3, 1)  # ch13:16 (line op)
        nc.sync.dma_start(out_v[:, 13:16, :],
                          arena[:, 11:14].rearrange("p c n x -> p c (n x)"))
        chD = pool.tile([P, 2, NB, 64], DT.uint8, name="chD")
        A.activation(chD[:, 0],
                     dgef[:, 2, :, None].broadcast_to((P, NB, 64)), Act.Copy)
        A.activation(chD[:, 1],
                     dgef[:, 3, :, None].broadcast_to((P, NB, 64)), Act.Copy)
        nc.sync.dma_start(out_v[:, 16:18, :],
                          chD.rearrange("p c n x -> p c (n x)"))


_NC_CACHE = {}


def _build_nc(allones):
    if allones in _NC_CACHE:
        return _NC_CACHE[allones]
    nc = bacc.Bacc("TRN2", debug=False, enable_asserts=False)
    state_d = nc.dram_tensor("state", [BPC, 64], DT.float32, kind="ExternalInput").ap()
    side_d = nc.dram_tensor("side", [BPC], DT.float32, kind="ExternalInput").ap()
    out_d = nc.dram_tensor("out", [18, BPC * 64], DT.uint8, kind="ExternalOutput").ap()
    with tile.TileContext(nc) as tc:
        feature_kernel(tc, out_d, state_d, side_d, allones)
    nc.finalize()
    _NC_CACHE[allones] = nc
    return nc


_JIT_CACHE = {}


def _get_runner(allones):
    if allones in _JIT_CACHE:
        return _JIT_CACHE[allones]
    import jax
    from jax.sharding import Mesh, PartitionSpec, NamedSharding
    try:
        from jax.experimental.shard_map import shard_map
    except ImportError:
        from jax.shard_map import shard_map  # newer jax
    from concourse import bass2jax as B2J

    B2J.install_neuronx_cc_hook()
    nc = _build_nc(allones)

    in_names = ["state", "side"]
    out_names = ["out"]
    out_avals = [jax.core.ShapedArray((18, BPC * 64), np.uint8)]
    all_names = in_names + out_names
    if nc.partition_id_tensor is not None:
        all_names = all_names + [nc.partition_id_tensor.name]

    def _body(state_a, side_a, zeros_a):
        operands = [state_a, side_a, zeros_a]
        if nc.partition_id_tensor is not None:
            operands.append(B2J.partition_id_tensor())
        outs = B2J._bass_exec_p.bind(
            *operands,
            out_avals=tuple(out_avals),
            in_names=tuple(all_names),
            out_names=tuple(out_names),
            lowering_input_output_aliases=(),
            sim_require_finite=True,
            sim_require_nnan=True,
            nc=nc,
        )
        return outs[0]

    devices = jax.devices()[:NCORES]
    mesh = Mesh(np.asarray(devices), ("core",))
    spec = PartitionSpec("core")
    sharded = jax.jit(
        shard_map(
            _body, mesh=mesh,
            in_specs=(spec, spec, spec),
            out_specs=spec,
            check_rep=False,
        ),
        donate_argnums=(2,),
        keep_unused=True,
    )

    def put(shards):
        arrs = [jax.device_put(s, devices[i]) for i, s in enumerate(shards)]
        global_shape = (sum(s.shape[0] for s in shards),) + shards[0].shape[1:]
        return jax.make_array_from_single_device_arrays(
            global_shape, NamedSharding(mesh, spec), arrs
        )

    _JIT_CACHE[allones] = (sharded, put)
    return _JIT_CACHE[allones]


def kernel(state, side):
    """Full-input entry point: state [32768,8,8] f32, side [32768] f32."""
    state = np.ascontiguousarray(np.asarray(state, dtype=np.float32)).reshape(-1, 64)
    side = np.ascontiguousarray(np.asarray(side, dtype=np.float32)).reshape(-1)
    B = state.shape[0]
    assert B == BPC * NCORES, (B, BPC * NCORES)
    allones = bool(np.all(side == 1.0))
    sharded, put = _get_runner(allones)
    state_g = put([state[i * BPC:(i + 1) * BPC] for i in range(NCORES)])
    side_g = put([side[i * BPC:(i + 1) * BPC] for i in range(NCORES)])
    zeros_g = put([np.zeros((18, BPC * 64), np.uint8) for _ in range(NCORES)])
    out = sharded(state_g, side_g, zeros_g)
    out = np.asarray(out).reshape(NCORES, 18, BPC, 8, 8)
    # device cell order is [j, r] (column-major); swap to [r, j]
    out = out.transpose(0, 2, 1, 4, 3).reshape(NCORES * BPC, 18, 8, 8)
    return out.astype(np.float32)
